# revision 20
# baseline (speedup 1.0000x reference)
"""Causal self-attention (dense transformer block) for 8 Trainium2 NeuronCores.

Sharding: DP over batch (2) x TP over heads (4 groups of 4 heads) = 8 cores.
Per core, a single pass over x computes QKV for all 4 heads (fp8 DoubleRow
matmuls at 4x f32r rate), RoPE, causal flash-attention (no-max softmax with
constant bias), then a row-parallel output projection producing a partial
[oc, t] result; the host sums the 4 TP partials per batch and transposes.

Precision plan (tolerance 2e-2 max-rel-err vs absmax):
 - fp8e4m3 (RNE casts) for the qk projection, S=qk^T, AV, L row-sum, and
   out-projection, with weights/q/k/v/y scaled by 16..32 to sit in e4m3's
   normal range.
 - bf16 insurance paths where early causal rows (concentrated attention)
   need accuracy: v-projection for keys 0..511, all of attention for query
   block 0 (S/AV/L in bf16), and the out-projection for rows 0..511.
 - The L row-sum uses an all-0.5 [128,2,128] fp8 stationary: DoubleRow-legal
   (M=128), broadcasts L across partitions (no gpsimd partition_broadcast),
   and pre-folds the 2x rescale of y8 = 32*y_true.

Self-contained: hardcodes shapes, builds/compiles/runs via
run_bass_kernel_spmd on cores 0-7.
"""

import os
import sys
import types

sys.path.insert(0, "/opt/trn_rl_repo")

import numpy as np
import ml_dtypes

import concourse.bass as bass
import concourse.mybir as mybir
import concourse.tile as tile
from concourse import bacc
from concourse.bass_utils import run_bass_kernel_spmd
from concourse.vector_clock import ScopedClock, VectorClock

F32 = mybir.dt.float32
BF16 = mybir.dt.bfloat16
FP8 = mybir.dt.float8e4
AF = mybir.ActivationFunctionType
DR = mybir.MatmulPerfMode.DoubleRow

P = 128
T = 2048
C = 2048
NH = 16          # total heads
HPC = 4          # heads per core
HSIZE = 128
N_CORES = 8
TG = 4           # t-groups of 512
QG = 512
SW = 16.0        # fp8 scale for w_qk, w_v, w_proj (=> q,k,v 16x-scaled)
SY = 32.0        # fp8 scale for y
EXP_BIAS = -2.0
SC8 = 1.0 / (float(np.sqrt(HSIZE)) * SW * SW)  # exp scale for 16x-scaled q,k

_TRACE = os.environ.get("BASS_KERNEL_TRACE", "0") == "1"

E4 = ml_dtypes.float8_e4m3
BF = ml_dtypes.bfloat16


def _patch_tile_drain():
    """walrus in this toolchain allows at most one sync-wait per instruction;
    TileContext's tail drain aggregates the whole global clock onto one Drain.
    Split it: one Drain per pending proc, each with a single wait."""
    if getattr(tile.TileContext, "_drain_patched", False):
        return

    def _drain_and_barrier(self, tick_clock, wait_clock):
        nc = self.nc
        gc = tick_clock.global_clock
        n = len(gc)
        for p in range(n):
            if gc[p] > 0:
                vc = VectorClock([gc[p] if i == p else 0 for i in range(n)])
                di = nc.sync.drain()
                wait_clock.add_sem_waits(di.ins, ScopedClock({None: vc}))
        nc.all_engine_barrier()
        popped = nc._tile_sem_poison_stack.pop()
        assert popped is self._sem_poison
        nc.clear_and_free_semaphores(list(self.sems.allocated().values()))
        nc.all_engine_barrier()

    tile.TileContext._drain_and_barrier = _drain_and_barrier
    tile.TileContext._drain_patched = True


def _install_ntff_hook():
    """Wire the axon NTFF profiling hook this image leaves unwired (the agent
    image's antenv lacks axon_hooks). Only needed when tracing."""
    import antenv

    if getattr(antenv, "axon_hooks", None) is not None:
        return
    mod = types.ModuleType("antenv.axon_hooks")
    mod._hook = None
    mod.set_axon_ntff_profile_hook = lambda h: setattr(mod, "_hook", h)
    mod.get_axon_ntff_profile_hook = lambda: mod._hook
    sys.modules["antenv.axon_hooks"] = mod
    antenv.axon_hooks = mod
    if "/root/.axon_site" not in sys.path:
        sys.path.insert(0, "/root/.axon_site")
    try:
        from trn_agent_boot.trn_boot import _ntff_profile_via_ctypes

        hook = _ntff_profile_via_ctypes("/opt/axon/libaxon_pjrt.so")
        if hook is not None:
            mod.set_axon_ntff_profile_hook(hook)
        import concourse.bass_utils as bu

        bu.upload_artifacts = lambda d: d
    except Exception:
        pass


def build_nc():
    _patch_tile_drain()
    nc = bacc.Bacc(None, target_bir_lowering=False)

    x8d = nc.dram_tensor("x8", [C, T], FP8, kind="ExternalInput")
    xbfd = nc.dram_tensor("xbf", [C, QG], BF16, kind="ExternalInput")
    w8d = nc.dram_tensor("w8", [C, 8 * HSIZE], FP8, kind="ExternalInput")
    wv8d = nc.dram_tensor("wv8", [C, 4 * HSIZE], FP8, kind="ExternalInput")
    wvbfd = nc.dram_tensor("wvbf", [C, 4 * HSIZE], BF16, kind="ExternalInput")
    wp8d = nc.dram_tensor("wp8", [4 * HSIZE, T], FP8, kind="ExternalInput")
    wpbfd = nc.dram_tensor("wpbf", [4 * HSIZE, T], BF16, kind="ExternalInput")
    c1d = nc.dram_tensor("c1", [P, T], BF16, kind="ExternalInput")
    c2d = nc.dram_tensor("c2", [P, T], BF16, kind="ExternalInput")
    mk8d = nc.dram_tensor("mk8", [4, P, QG], FP8, kind="ExternalInput")
    onesdrd = nc.dram_tensor("onesdr", [P, 2, P], FP8, kind="ExternalInput")
    onesbfd = nc.dram_tensor("onesbf", [P, P], BF16, kind="ExternalInput")
    swp8d = nc.dram_tensor("swp8", [P, P], FP8, kind="ExternalInput")
    swpbfd = nc.dram_tensor("swpbf", [P, P], BF16, kind="ExternalInput")
    wqkbfd = nc.dram_tensor("wqkbf", [C, 8 * HSIZE], BF16, kind="ExternalInput")
    outT = nc.dram_tensor("outT", [T, T], BF16, kind="ExternalOutput")

    x8r = x8d.rearrange("(cc p) t -> p cc t", p=P)       # [128,16,2048]
    xbfr = xbfd.rearrange("(cc p) t -> p cc t", p=P)     # [128,16,512]
    w8r = w8d.rearrange("(cc p) j -> p cc j", p=P)       # [128,16,1024]
    wv8r = wv8d.rearrange("(cc p) j -> p cc j", p=P)     # [128,16,512]
    wvbfr = wvbfd.rearrange("(cc p) j -> p cc j", p=P)
    wp8r = wp8d.rearrange("(hc p) t -> p hc t", p=P)     # [128,4,2048]
    wpbfr = wpbfd.rearrange("(hc p) t -> p hc t", p=P)
    mk8r = mk8d.rearrange("s p q -> p s q")              # [128,4,512]
    wqkbfr = wqkbfd.rearrange("(cc p) j -> p cc j", p=P)

    with tile.TileContext(nc) as tc, nc.allow_low_precision(
        reason="fp8/bf16 storage are the intended reduced-precision formats"
    ):
        with (
            tc.tile_pool(name="const", bufs=1) as constp,
            tc.tile_pool(name="big", bufs=1) as bigp,
            tc.tile_pool(name="wk8", bufs=3) as wk8,      # qraw fp8
            tc.tile_pool(name="wkb", bufs=5) as wkb,      # t1/t2 bf16
            tc.tile_pool(name="pp8", bufs=3) as pp8,      # p pairs fp8
            tc.tile_pool(name="ppb", bufs=5) as ppb,      # p singles bf16
            tc.tile_pool(name="rp", bufs=2) as rpool,     # r128 f32
            tc.tile_pool(name="st", bufs=4) as stp,       # outproj stage bf16
            tc.tile_pool(name="mm", bufs=4, space="PSUM") as mmp,
            tc.tile_pool(name="yt", bufs=2, space="PSUM") as ytp,
            tc.tile_pool(name="lp", bufs=2, space="PSUM") as lpp,
        ):
            # ---- constants / weights ----
            swp8 = constp.tile([P, P], FP8, tag="swp8")
            onesdr = constp.tile([P, 2, P], FP8, tag="onesdr")
            onesbf = constp.tile([P, P], BF16, tag="onesbf")
            c1 = constp.tile([P, T], BF16, tag="c1")
            c2 = constp.tile([P, T], BF16, tag="c2")
            mk8 = constp.tile([P, 4, QG], FP8, tag="mk8")
            ebias = constp.tile([P, 1], F32, tag="ebias")
            swpbf = constp.tile([P, P], BF16, tag="swpbf")
            nc.gpsimd.memset(ebias[:], EXP_BIAS)
            nc.sync.dma_start(swp8[:], swp8d[:])
            nc.sync.dma_start(onesdr[:], onesdrd[:])
            nc.sync.dma_start(onesbf[:], onesbfd[:])

            x8sb = bigp.tile([P, 16, T], FP8, tag="x8")
            nc.sync.dma_start(x8sb[:, :, QG:2 * QG], x8r[:, :, QG:2 * QG])
            w8sb = bigp.tile([P, 16, 8 * HSIZE], FP8, tag="w8")
            for cc in range(4):
                nc.sync.dma_start(
                    w8sb[:, 4 * cc:4 * cc + 4, :], w8r[:, 4 * cc:4 * cc + 4, :]
                )
            nc.sync.dma_start(c1[:], c1d[:])
            nc.sync.dma_start(c2[:], c2d[:])
            wv8sb = bigp.tile([P, 16, 4 * HSIZE], FP8, tag="wv8")
            nc.sync.dma_start(wv8sb[:], wv8r)
            for tg in (2, 3, 0):
                nc.sync.dma_start(
                    x8sb[:, :, tg * QG:(tg + 1) * QG],
                    x8r[:, :, tg * QG:(tg + 1) * QG],
                )
            nc.sync.dma_start(mk8[:], mk8r)
            xbf = bigp.tile([P, 16, QG], BF16, tag="xbf")
            nc.sync.dma_start(xbf[:], xbfr)
            wvbf = bigp.tile([P, 16, 4 * HSIZE], BF16, tag="wvbf")
            nc.sync.dma_start(wvbf[:], wvbfr)
            wqkbf = bigp.tile([P, 16, 4 * HSIZE], BF16, tag="wqkbf")
            nc.sync.dma_start(wqkbf[:], wqkbfr[:, :, 0:4 * HSIZE])
            nc.sync.dma_start(swpbf[:], swpbfd[:])
            wp8sb = bigp.tile([P, 4, T], FP8, tag="wp8")
            nc.sync.dma_start(wp8sb[:], wp8r)
            wpbf = bigp.tile([P, 4, T], BF16, tag="wpbf")
            nc.sync.dma_start(wpbf[:], wpbfr)

            # persistent activations
            q8 = [bigp.tile([P, T], FP8, tag=f"q8_{h}", name=f"q8_{h}") for h in range(4)]
            k8 = [bigp.tile([P, T], FP8, tag=f"k8_{h}", name=f"k8_{h}") for h in range(4)]
            qkbf = bigp.tile([P, 8, QG], BF16, tag="qkbf")  # tg0 q(0:4)/k(4:8)
            v8 = bigp.tile([P, 16, 4 * HSIZE], FP8, tag="v8")
            vbf = bigp.tile([P, 4, 4 * HSIZE], BF16, tag="vbf")
            y8t = [bigp.tile([P, 4, QG], FP8, tag=f"y8_{i}", name=f"y8_{i}") for i in range(3)]
            ybf = bigp.tile([P, 4, QG], BF16, tag="ybf")

            # PE warmup while first DMAs land (un-throttle HAM)
            ps_wu = mmp.tile([P, QG], F32, tag="mm", name="ps_wu")
            for _ in range(56):
                nc.tensor.matmul(
                    ps_wu[:, 0:P], swp8[:], onesdr[:, 0, :], start=True, stop=True
                )

            # ---- QKV: one pass over x for all 4 heads; tg0 last so the
            # bf16 insurance operands (xbf/wvbf) have time to arrive ----
            for tg in (1, 2, 3, 0):
                tsl = slice(tg * QG, (tg + 1) * QG)
                for j in range(8):  # 0-3: q heads, 4-7: k heads
                    psq = mmp.tile([P, QG], F32, tag="mm", name=f"psq{j}")
                    for cc in range(8):
                        nc.tensor.matmul(
                            psq[:],
                            w8sb[:, 2 * cc:2 * cc + 2, j * P:(j + 1) * P],
                            x8sb[:, 2 * cc:2 * cc + 2, tsl],
                            start=(cc == 0),
                            stop=(cc == 7),
                            perf_mode=DR,
                        )
                    # RoPE: dst = psq*c1 + swap64(psq)*c2
                    qraw = wk8.tile([P, QG], FP8, tag="qraw", name="qraw")
                    nc.scalar.activation(qraw[:], psq[:], AF.Copy)
                    ps_sw = mmp.tile([P, QG], F32, tag="mm", name="ps_sw")
                    nc.tensor.matmul(ps_sw[:], swp8[:], qraw[:], start=True, stop=True)
                    t1 = wkb.tile([P, QG], BF16, tag="t", name="t1")
                    t2 = wkb.tile([P, QG], BF16, tag="t", name="t2")
                    nc.vector.tensor_mul(t1[:], psq[:], c1[:, tsl])
                    nc.vector.tensor_mul(t2[:], ps_sw[:], c2[:, tsl])
                    dst = (q8[j] if j < 4 else k8[j - 4])[:, tsl]
                    nc.vector.tensor_add(dst, t1[:], t2[:])
                    if tg == 0:
                        nc.vector.tensor_add(qkbf[:, j, :], t1[:], t2[:])
                # v projection for this tg's 4 key tiles
                for tt in range(4):
                    kt = tg * 4 + tt
                    psv = ytp.tile([P, QG], F32, tag="yt", name="psv")
                    for cc in range(8):
                        nc.tensor.matmul(
                            psv[:],
                            x8sb[:, 2 * cc:2 * cc + 2, kt * P:(kt + 1) * P],
                            wv8sb[:, 2 * cc:2 * cc + 2, :],
                            start=(cc == 0),
                            stop=(cc == 7),
                            perf_mode=DR,
                        )
                    nc.scalar.copy(v8[:, kt, :], psv[:])

            def vbf_tile(tt):
                psv = ytp.tile([P, QG], F32, tag="yt", name="psvb")
                for cc in range(16):
                    nc.tensor.matmul(
                        psv[:],
                        xbf[:, cc, tt * P:(tt + 1) * P],
                        wvbf[:, cc, :],
                        start=(cc == 0),
                        stop=(cc == 15),
                    )
                nc.scalar.copy(vbf[:, tt, :], psv[:])

            # ---- bf16 re-projection of q,k for t<128: every logit of the
            # concentrated early rows (<128) then carries only bf16 noise ----
            def reproj(j):
                if j == 4:  # second chunk of wqkbf replaces the first
                    nc.sync.dma_start(wqkbf[:], wqkbfr[:, :, 4 * HSIZE:8 * HSIZE])
                psq2 = mmp.tile([P, QG], F32, tag="mm", name="psq2")
                for cc in range(16):
                    nc.tensor.matmul(
                        psq2[:, 0:P],
                        wqkbf[:, cc, (j % 4) * P:(j % 4 + 1) * P],
                        xbf[:, cc, 0:P],
                        start=(cc == 0),
                        stop=(cc == 15),
                    )
                qraw2 = wkb.tile([P, QG], BF16, tag="t", name="qraw2")
                nc.scalar.activation(qraw2[:, 0:P], psq2[:, 0:P], AF.Copy)
                ps_sw2 = mmp.tile([P, QG], F32, tag="mm", name="ps_sw2")
                nc.tensor.matmul(
                    ps_sw2[:, 0:P], swpbf[:], qraw2[:, 0:P], start=True, stop=True
                )
                t1b = wkb.tile([P, QG], BF16, tag="t", name="t1b")
                t2b = wkb.tile([P, QG], BF16, tag="t", name="t2b")
                nc.vector.tensor_mul(t1b[:, 0:P], psq2[:, 0:P], c1[:, 0:P])
                nc.vector.tensor_mul(t2b[:, 0:P], ps_sw2[:, 0:P], c2[:, 0:P])
                nc.vector.tensor_add(qkbf[:, j, 0:P], t1b[:, 0:P], t2b[:, 0:P])

            # ---- attention + interleaved output projection ----
            pending_norm = []

            def emit_norm():
                h_, qg_, ps_y_, ps_l_ = pending_norm.pop(0)
                r128 = rpool.tile([P, QG], F32, tag="r", name="r128")
                nc.vector.reciprocal_approx_fast(r128[:], ps_l_[:])
                if qg_ == 0:
                    nc.vector.tensor_mul(ybf[:, h_, :], ps_y_[:], r128[:])
                else:
                    nc.vector.tensor_mul(
                        y8t[qg_ - 1][:, h_, :], ps_y_[:], r128[:]
                    )

            def emit_outproj_tile(tg_, oc):
                ps_o = mmp.tile([P, QG], F32, tag="mm", name="ps_o")
                if tg_ == 0:
                    for hc in range(4):
                        nc.tensor.matmul(
                            ps_o[:],
                            wpbf[:, hc, oc * P:(oc + 1) * P],
                            ybf[:, hc, :],
                            start=(hc == 0),
                            stop=(hc == 3),
                        )
                else:
                    for i in range(2):
                        nc.tensor.matmul(
                            ps_o[:],
                            wp8sb[:, 2 * i:2 * i + 2, oc * P:(oc + 1) * P],
                            y8t[tg_ - 1][:, 2 * i:2 * i + 2, :],
                            start=(i == 0),
                            stop=(i == 1),
                            perf_mode=DR,
                        )
                stage = stp.tile([P, QG], BF16, tag="s", name="stage")
                if tg_ == 0:
                    nc.vector.tensor_copy(stage[:], ps_o[:])
                else:
                    nc.vector.tensor_scalar_mul(stage[:], ps_o[:], 1.0 / (SY * SW))
                nc.sync.dma_start(
                    outT[oc * P:(oc + 1) * P, tg_ * QG:(tg_ + 1) * QG], stage[:]
                )

            op_queue = []  # deferred outproj tiles, emitted during next qg

            def attn(h, qg):
                n_kt = 4 * (qg + 1)
                qsl = slice(qg * QG, (qg + 1) * QG)
                ps_y = ytp.tile([P, QG], F32, tag="yt", name="ps_y")
                ps_l = lpp.tile([P, QG], F32, tag="l", name="ps_l")
                LA = 3
                p_cur = {}   # fp8: pair index -> p2 tile; bf16: kt -> tile

                def emit_s(kt):
                    ps_s = mmp.tile([P, QG], F32, tag="mm", name="ps_s")
                    if qg == 0:
                        nc.tensor.matmul(
                            ps_s[:],
                            qkbf[:, 4 + h, kt * P:(kt + 1) * P],
                            qkbf[:, h, :],
                            start=True,
                            stop=True,
                        )
                        dst = ppb.tile([P, QG], BF16, tag="pb", name="pb")
                        p_cur[kt] = dst
                        dsl = dst[:]
                    else:
                        if kt % 2 == 0:
                            p_cur[kt // 2] = pp8.tile(
                                [P, 2, QG], FP8, tag="p2", name="p2"
                            )
                        nc.tensor.matmul(
                            ps_s[:],
                            k8[h][:, kt * P:(kt + 1) * P],
                            q8[h][:, qsl],
                            start=True,
                            stop=True,
                        )
                        dsl = p_cur[kt // 2][:, kt % 2, :]
                    nc.scalar.activation(dsl, ps_s[:], AF.Exp, bias=ebias[:], scale=SC8)
                    if kt >= 4 * qg:
                        s = kt - 4 * qg
                        nc.vector.tensor_mul(dsl, dsl, mk8[:, s, :])

                def emit_av(t):
                    if qg == 0:
                        p = p_cur.pop(t)
                        nc.tensor.matmul(
                            ps_y[:],
                            vbf[:, t, h * P:(h + 1) * P],
                            p[:],
                            start=(t == 0),
                            stop=(t == n_kt - 1),
                        )
                        nc.tensor.matmul(
                            ps_l[:],
                            onesbf[:],
                            p[:],
                            start=(t == 0),
                            stop=(t == n_kt - 1),
                        )
                    else:
                        if t % 2 == 0:
                            return
                        pr = t // 2
                        p2 = p_cur.pop(pr)
                        nc.tensor.matmul(
                            ps_y[:],
                            v8[:, 2 * pr:2 * pr + 2, h * P:(h + 1) * P],
                            p2[:],
                            start=(pr == 0),
                            stop=(t == n_kt - 1),
                            perf_mode=DR,
                        )
                        nc.tensor.matmul(
                            ps_l[:],
                            onesdr[:],
                            p2[:],
                            start=(pr == 0),
                            stop=(t == n_kt - 1),
                            perf_mode=DR,
                        )

                for step in range(n_kt + LA):
                    if step < n_kt:
                        emit_s(step)
                    if step == 2 and pending_norm:
                        emit_norm()
                    if step >= LA:
                        emit_av(step - LA)
                pending_norm.append((h, qg, ps_y, ps_l))

            op_queue.extend((lambda t_=t: vbf_tile(t_)) for t in range(4))
            op_queue.extend((lambda j_=j: reproj(j_)) for j in range(8))
            for qg in (1, 2, 0, 3):
                for h in range(4):
                    attn(h, qg)
                    for _ in range(6):
                        if op_queue:
                            op_queue.pop(0)()
                while pending_norm:
                    emit_norm()
                g = qg
                op_queue.extend(
                    (lambda tg_=g, oc_=oc: emit_outproj_tile(tg_, oc_))
                    for oc in range(16)
                )
            while op_queue:
                op_queue.pop(0)()

    nc.finalize()
    return nc


def _host_inputs(x, freqs_cis, w_attn, w_proj):
    """Build the 8 per-core input maps."""
    x = np.asarray(x, dtype=np.float32)
    freqs_cis = np.asarray(freqs_cis, dtype=np.float32)
    w_attn = np.asarray(w_attn, dtype=np.float32)
    w_proj = np.asarray(w_proj, dtype=np.float32)

    B = x.shape[0]
    perm = np.concatenate([np.arange(0, HSIZE, 2), np.arange(1, HSIZE, 2)])

    cos = np.ascontiguousarray(freqs_cis[:, :, 0].T)  # [64, T]
    sin = np.ascontiguousarray(freqs_cis[:, :, 1].T)
    c1 = np.concatenate([cos, cos], axis=0).astype(BF)   # [128, T]
    c2 = np.concatenate([-sin, sin], axis=0).astype(BF)

    kk = np.arange(P)[:, None]
    ccol = np.arange(QG)[None, :]
    mk = np.stack(
        [(ccol >= s * 128 + kk).astype(np.float32) for s in range(4)], axis=0
    )  # [4,128,512]
    mk8 = mk.astype(E4)

    onesdr = (np.full((P, 2, P), 0.5, np.float32)).astype(E4)
    onesbf = np.ones((P, P), np.float32).astype(BF)
    swp = np.zeros((P, P), np.float32)
    for m in range(P):
        swp[(m + 64) % P, m] = 1.0
    swp8 = swp.astype(E4)
    swpbf_h = swp.astype(BF)

    xT = [np.ascontiguousarray(x[b].T) for b in range(B)]
    x8 = [t.astype(E4) for t in xT]
    xbf = [np.ascontiguousarray(t[:, :QG]).astype(BF) for t in xT]

    in_maps = []
    for core in range(N_CORES):
        b, g = core // 4, core % 4
        qk_blocks = []
        for off in (0, C):  # q then k
            for hh in range(4):
                hglob = 4 * g + hh
                cols = w_attn[:, off + hglob * HSIZE: off + (hglob + 1) * HSIZE]
                qk_blocks.append(cols[:, perm])
        wqk = np.concatenate(qk_blocks, axis=1)          # [C, 1024]
        w8 = (SW * wqk).astype(E4)
        wqkbf_h = (SW * wqk).astype(BF)
        wv = w_attn[:, 2 * C + 4 * g * HSIZE: 2 * C + 4 * (g + 1) * HSIZE]
        wv8 = (SW * wv).astype(E4)
        wvbf = wv.astype(BF)
        wp = w_proj[g * 512:(g + 1) * 512, :]
        wp8 = (SW * wp).astype(E4)
        wpbf = wp.astype(BF)
        in_maps.append(
            {
                "x8": x8[b],
                "xbf": xbf[b],
                "w8": np.ascontiguousarray(w8),
                "wqkbf": np.ascontiguousarray(wqkbf_h),
                "wv8": np.ascontiguousarray(wv8),
                "wvbf": np.ascontiguousarray(wvbf),
                "wp8": np.ascontiguousarray(wp8),
                "wpbf": np.ascontiguousarray(wpbf),
                "c1": c1,
                "c2": c2,
                "mk8": mk8,
                "onesdr": onesdr,
                "onesbf": onesbf,
                "swp8": swp8,
                "swpbf": swpbf_h,
            }
        )
    return in_maps


_LAST_RESULT = {}


def kernel(x, freqs_cis, w_attn, w_proj):
    if _TRACE:
        _install_ntff_hook()
    in_maps = _host_inputs(x, freqs_cis, w_attn, w_proj)
    nc = build_nc()
    res = run_bass_kernel_spmd(
        nc, in_maps, core_ids=list(range(N_CORES)), trace=_TRACE
    )
    _LAST_RESULT["res"] = res

    B = x.shape[0]
    out = np.zeros((B, T, C), dtype=np.float32)
    for core in range(N_CORES):
        b = core // 4
        out[b] += np.asarray(res.results[core]["outT"]).astype(np.float32).T
    return out


# revision 21
# speedup vs baseline: 1.1170x; 1.1170x over previous
"""Causal self-attention (dense transformer block) for 8 Trainium2 NeuronCores.

Sharding: DP over batch (2) x TP over heads (4 groups of 4 heads) = 8 cores.
Per core, a single pass over x computes QKV for all 4 heads (fp8 DoubleRow
matmuls at 4x f32r rate), RoPE, causal flash-attention (no-max softmax with
constant bias), then a row-parallel output projection producing a partial
[oc, t] result; the host sums the 4 TP partials per batch and transposes.

Precision plan (tolerance 2e-2 max-rel-err vs absmax):
 - fp8e4m3 (RNE casts) for the qk projection, S=qk^T, AV, L row-sum, and
   out-projection, with weights/q/k/v/y scaled by 16..32 to sit in e4m3's
   normal range.
 - bf16 insurance paths where early causal rows (concentrated attention)
   need accuracy: v-projection for keys 0..511, all of attention for query
   block 0 (S/AV/L in bf16), and the out-projection for rows 0..511.
 - The L row-sum uses an all-0.5 [128,2,128] fp8 stationary: DoubleRow-legal
   (M=128), broadcasts L across partitions (no gpsimd partition_broadcast),
   and pre-folds the 2x rescale of y8 = 32*y_true.

Self-contained: hardcodes shapes, builds/compiles/runs via
run_bass_kernel_spmd on cores 0-7.
"""

import os
import sys
import types

sys.path.insert(0, "/opt/trn_rl_repo")

import numpy as np
import ml_dtypes

import concourse.bass as bass
import concourse.mybir as mybir
import concourse.tile as tile
from concourse import bacc
from concourse.bass_utils import run_bass_kernel_spmd
from concourse.vector_clock import ScopedClock, VectorClock

F32 = mybir.dt.float32
BF16 = mybir.dt.bfloat16
FP8 = mybir.dt.float8e4
AF = mybir.ActivationFunctionType
DR = mybir.MatmulPerfMode.DoubleRow

P = 128
T = 2048
C = 2048
NH = 16          # total heads
HPC = 4          # heads per core
HSIZE = 128
N_CORES = 8
TG = 4           # t-groups of 512
QG = 512
SW = 16.0        # fp8 scale for w_qk, w_v, w_proj (=> q,k,v 16x-scaled)
SY = 32.0        # fp8 scale for y
EXP_BIAS = -2.0
SC8 = 1.0 / (float(np.sqrt(HSIZE)) * SW * SW)  # exp scale for 16x-scaled q,k

_TRACE = os.environ.get("BASS_KERNEL_TRACE", "0") == "1"

E4 = ml_dtypes.float8_e4m3
BF = ml_dtypes.bfloat16


def _patch_tile_drain():
    """walrus in this toolchain allows at most one sync-wait per instruction;
    TileContext's tail drain aggregates the whole global clock onto one Drain.
    Split it: one Drain per pending proc, each with a single wait."""
    if getattr(tile.TileContext, "_drain_patched", False):
        return

    def _drain_and_barrier(self, tick_clock, wait_clock):
        nc = self.nc
        gc = tick_clock.global_clock
        n = len(gc)
        for p in range(n):
            if gc[p] > 0:
                vc = VectorClock([gc[p] if i == p else 0 for i in range(n)])
                di = nc.sync.drain()
                wait_clock.add_sem_waits(di.ins, ScopedClock({None: vc}))
        nc.all_engine_barrier()
        popped = nc._tile_sem_poison_stack.pop()
        assert popped is self._sem_poison
        nc.clear_and_free_semaphores(list(self.sems.allocated().values()))
        nc.all_engine_barrier()

    tile.TileContext._drain_and_barrier = _drain_and_barrier
    tile.TileContext._drain_patched = True


def _install_ntff_hook():
    """Wire the axon NTFF profiling hook this image leaves unwired (the agent
    image's antenv lacks axon_hooks). Only needed when tracing."""
    import antenv

    if getattr(antenv, "axon_hooks", None) is not None:
        return
    mod = types.ModuleType("antenv.axon_hooks")
    mod._hook = None
    mod.set_axon_ntff_profile_hook = lambda h: setattr(mod, "_hook", h)
    mod.get_axon_ntff_profile_hook = lambda: mod._hook
    sys.modules["antenv.axon_hooks"] = mod
    antenv.axon_hooks = mod
    if "/root/.axon_site" not in sys.path:
        sys.path.insert(0, "/root/.axon_site")
    try:
        from trn_agent_boot.trn_boot import _ntff_profile_via_ctypes

        hook = _ntff_profile_via_ctypes("/opt/axon/libaxon_pjrt.so")
        if hook is not None:
            mod.set_axon_ntff_profile_hook(hook)
        import concourse.bass_utils as bu

        bu.upload_artifacts = lambda d: d
    except Exception:
        pass


def build_nc():
    _patch_tile_drain()
    nc = bacc.Bacc(None, target_bir_lowering=False)

    x8d = nc.dram_tensor("x8", [C, T], FP8, kind="ExternalInput")
    xbfd = nc.dram_tensor("xbf", [C, QG], BF16, kind="ExternalInput")
    w8d = nc.dram_tensor("w8", [C, 8 * HSIZE], FP8, kind="ExternalInput")
    wv8d = nc.dram_tensor("wv8", [C, 4 * HSIZE], FP8, kind="ExternalInput")
    wvbfd = nc.dram_tensor("wvbf", [C, 4 * HSIZE], BF16, kind="ExternalInput")
    wp8d = nc.dram_tensor("wp8", [4 * HSIZE, T], FP8, kind="ExternalInput")
    wpbfd = nc.dram_tensor("wpbf", [4 * HSIZE, T], BF16, kind="ExternalInput")
    c1d = nc.dram_tensor("c1", [P, T], BF16, kind="ExternalInput")
    c2d = nc.dram_tensor("c2", [P, T], BF16, kind="ExternalInput")
    mk8d = nc.dram_tensor("mk8", [4, P, QG], FP8, kind="ExternalInput")
    onesdrd = nc.dram_tensor("onesdr", [P, 2, P], FP8, kind="ExternalInput")
    onesbfd = nc.dram_tensor("onesbf", [P, P], BF16, kind="ExternalInput")
    swp8d = nc.dram_tensor("swp8", [P, P], FP8, kind="ExternalInput")
    swpbfd = nc.dram_tensor("swpbf", [P, P], BF16, kind="ExternalInput")
    wqkbfd = nc.dram_tensor("wqkbf", [C, 8 * HSIZE], BF16, kind="ExternalInput")
    outT = nc.dram_tensor("outT", [T, T], BF16, kind="ExternalOutput")

    x8r = x8d.rearrange("(cc p) t -> p cc t", p=P)       # [128,16,2048]
    xbfr = xbfd.rearrange("(cc p) t -> p cc t", p=P)     # [128,16,512]
    w8r = w8d.rearrange("(cc p) j -> p cc j", p=P)       # [128,16,1024]
    wv8r = wv8d.rearrange("(cc p) j -> p cc j", p=P)     # [128,16,512]
    wvbfr = wvbfd.rearrange("(cc p) j -> p cc j", p=P)
    wp8r = wp8d.rearrange("(hc p) t -> p hc t", p=P)     # [128,4,2048]
    wpbfr = wpbfd.rearrange("(hc p) t -> p hc t", p=P)
    mk8r = mk8d.rearrange("s p q -> p s q")              # [128,4,512]
    wqkbfr = wqkbfd.rearrange("(cc p) j -> p cc j", p=P)

    with tile.TileContext(nc) as tc, nc.allow_low_precision(
        reason="fp8/bf16 storage are the intended reduced-precision formats"
    ):
        with (
            tc.tile_pool(name="const", bufs=1) as constp,
            tc.tile_pool(name="big", bufs=1) as bigp,
            tc.tile_pool(name="wk8", bufs=3) as wk8,      # qraw fp8
            tc.tile_pool(name="wkb", bufs=5) as wkb,      # t1/t2 bf16
            tc.tile_pool(name="pp8", bufs=3) as pp8,      # p pairs fp8
            tc.tile_pool(name="ppb", bufs=5) as ppb,      # p singles bf16
            tc.tile_pool(name="rp", bufs=2) as rpool,     # r128 f32
            tc.tile_pool(name="st", bufs=4) as stp,       # outproj stage bf16
            tc.tile_pool(name="mm", bufs=4, space="PSUM") as mmp,
            tc.tile_pool(name="yt", bufs=2, space="PSUM") as ytp,
            tc.tile_pool(name="lp", bufs=2, space="PSUM") as lpp,
        ):
            # ---- constants / weights ----
            swp8 = constp.tile([P, P], FP8, tag="swp8")
            onesdr = constp.tile([P, 2, P], FP8, tag="onesdr")
            onesbf = constp.tile([P, P], BF16, tag="onesbf")
            c1 = constp.tile([P, T], BF16, tag="c1")
            c2 = constp.tile([P, T], BF16, tag="c2")
            mk8 = constp.tile([P, 4, QG], FP8, tag="mk8")
            ebias = constp.tile([P, 1], F32, tag="ebias")
            swpbf = constp.tile([P, P], BF16, tag="swpbf")
            nc.gpsimd.memset(ebias[:], EXP_BIAS)
            nc.sync.dma_start(swp8[:], swp8d[:])
            nc.sync.dma_start(onesdr[:], onesdrd[:])
            nc.sync.dma_start(onesbf[:], onesbfd[:])

            x8sb = bigp.tile([P, 16, T], FP8, tag="x8")
            nc.sync.dma_start(x8sb[:, :, QG:2 * QG], x8r[:, :, QG:2 * QG])
            w8sb = bigp.tile([P, 16, 8 * HSIZE], FP8, tag="w8")
            for cc in range(4):
                nc.sync.dma_start(
                    w8sb[:, 4 * cc:4 * cc + 4, :], w8r[:, 4 * cc:4 * cc + 4, :]
                )
            nc.sync.dma_start(c1[:], c1d[:])
            nc.sync.dma_start(c2[:], c2d[:])
            wv8sb = bigp.tile([P, 16, 4 * HSIZE], FP8, tag="wv8")
            nc.sync.dma_start(wv8sb[:], wv8r)
            for tg in (2, 3, 0):
                nc.sync.dma_start(
                    x8sb[:, :, tg * QG:(tg + 1) * QG],
                    x8r[:, :, tg * QG:(tg + 1) * QG],
                )
            nc.sync.dma_start(mk8[:], mk8r)
            xbf = bigp.tile([P, 16, QG], BF16, tag="xbf")
            nc.sync.dma_start(xbf[:], xbfr)
            wvbf = bigp.tile([P, 16, 4 * HSIZE], BF16, tag="wvbf")
            nc.sync.dma_start(wvbf[:], wvbfr)
            wqkbf = bigp.tile([P, 16, 4 * HSIZE], BF16, tag="wqkbf")
            nc.sync.dma_start(wqkbf[:], wqkbfr[:, :, 0:4 * HSIZE])
            nc.sync.dma_start(swpbf[:], swpbfd[:])
            wp8sb = bigp.tile([P, 4, T], FP8, tag="wp8")
            nc.sync.dma_start(wp8sb[:], wp8r)
            wpbf = bigp.tile([P, 4, T], BF16, tag="wpbf")
            nc.sync.dma_start(wpbf[:], wpbfr)

            # persistent activations
            q8 = [bigp.tile([P, T], FP8, tag=f"q8_{h}", name=f"q8_{h}") for h in range(4)]
            k8 = [bigp.tile([P, T], FP8, tag=f"k8_{h}", name=f"k8_{h}") for h in range(4)]
            qkbf = bigp.tile([P, 8, QG], BF16, tag="qkbf")  # tg0 q(0:4)/k(4:8)
            v8 = bigp.tile([P, 16, 4 * HSIZE], FP8, tag="v8")
            vbf = bigp.tile([P, 4, 4 * HSIZE], BF16, tag="vbf")
            y8t = [bigp.tile([P, 4, QG], FP8, tag=f"y8_{i}", name=f"y8_{i}") for i in range(3)]
            ybf = bigp.tile([P, 4, QG], BF16, tag="ybf")

            # PE warmup while first DMAs land (un-throttle HAM)
            ps_wu = mmp.tile([P, QG], F32, tag="mm", name="ps_wu")
            for _ in range(56):
                nc.tensor.matmul(
                    ps_wu[:, 0:P], swp8[:], onesdr[:, 0, :], start=True, stop=True
                )

            # ---- QKV: one pass over x for all 4 heads; tg0 last so the
            # bf16 insurance operands (xbf/wvbf) have time to arrive ----
            for tg in (1, 2, 3, 0):
                tsl = slice(tg * QG, (tg + 1) * QG)
                for j in range(8):  # 0-3: q heads, 4-7: k heads
                    psq = mmp.tile([P, QG], F32, tag="mm", name=f"psq{j}")
                    for cc in range(8):
                        nc.tensor.matmul(
                            psq[:],
                            w8sb[:, 2 * cc:2 * cc + 2, j * P:(j + 1) * P],
                            x8sb[:, 2 * cc:2 * cc + 2, tsl],
                            start=(cc == 0),
                            stop=(cc == 7),
                            perf_mode=DR,
                        )
                    # RoPE: dst = psq*c1 + swap64(psq)*c2
                    qraw = wk8.tile([P, QG], FP8, tag="qraw", name="qraw")
                    nc.scalar.activation(qraw[:], psq[:], AF.Copy)
                    ps_sw = mmp.tile([P, QG], F32, tag="mm", name="ps_sw")
                    nc.tensor.matmul(ps_sw[:], swp8[:], qraw[:], start=True, stop=True)
                    t1 = wkb.tile([P, QG], BF16, tag="t", name="t1")
                    t2 = wkb.tile([P, QG], BF16, tag="t", name="t2")
                    nc.vector.tensor_mul(t1[:], psq[:], c1[:, tsl])
                    nc.vector.tensor_mul(t2[:], ps_sw[:], c2[:, tsl])
                    dst = (q8[j] if j < 4 else k8[j - 4])[:, tsl]
                    nc.vector.tensor_add(dst, t1[:], t2[:])
                    if tg == 0:
                        nc.vector.tensor_add(qkbf[:, j, :], t1[:], t2[:])
                # v projection for this tg's 4 key tiles
                for tt in range(4):
                    kt = tg * 4 + tt
                    psv = ytp.tile([P, QG], F32, tag="yt", name="psv")
                    if tg == 0:
                        for cc in range(16):
                            nc.tensor.matmul(
                                psv[:],
                                xbf[:, cc, tt * P:(tt + 1) * P],
                                wvbf[:, cc, :],
                                start=(cc == 0),
                                stop=(cc == 15),
                            )
                        nc.scalar.copy(vbf[:, tt, :], psv[:])
                        nc.scalar.mul(v8[:, kt, :], psv[:], SW)
                    else:
                        for cc in range(8):
                            nc.tensor.matmul(
                                psv[:],
                                x8sb[:, 2 * cc:2 * cc + 2, kt * P:(kt + 1) * P],
                                wv8sb[:, 2 * cc:2 * cc + 2, :],
                                start=(cc == 0),
                                stop=(cc == 7),
                                perf_mode=DR,
                            )
                        nc.scalar.copy(v8[:, kt, :], psv[:])

            # ---- bf16 re-projection of q,k for t<128: every logit of the
            # concentrated early rows (<128) then carries only bf16 noise ----
            for j in range(8):
                if j == 4:  # second chunk of wqkbf replaces the first
                    nc.sync.dma_start(wqkbf[:], wqkbfr[:, :, 4 * HSIZE:8 * HSIZE])
                psq2 = mmp.tile([P, QG], F32, tag="mm", name="psq2")
                for cc in range(16):
                    nc.tensor.matmul(
                        psq2[:, 0:P],
                        wqkbf[:, cc, (j % 4) * P:(j % 4 + 1) * P],
                        xbf[:, cc, 0:P],
                        start=(cc == 0),
                        stop=(cc == 15),
                    )
                qraw2 = wkb.tile([P, QG], BF16, tag="t", name="qraw2")
                nc.scalar.activation(qraw2[:, 0:P], psq2[:, 0:P], AF.Copy)
                ps_sw2 = mmp.tile([P, QG], F32, tag="mm", name="ps_sw2")
                nc.tensor.matmul(
                    ps_sw2[:, 0:P], swpbf[:], qraw2[:, 0:P], start=True, stop=True
                )
                t1b = wkb.tile([P, QG], BF16, tag="t", name="t1b")
                t2b = wkb.tile([P, QG], BF16, tag="t", name="t2b")
                nc.vector.tensor_mul(t1b[:, 0:P], psq2[:, 0:P], c1[:, 0:P])
                nc.vector.tensor_mul(t2b[:, 0:P], ps_sw2[:, 0:P], c2[:, 0:P])
                nc.vector.tensor_add(qkbf[:, j, 0:P], t1b[:, 0:P], t2b[:, 0:P])

            # ---- attention + interleaved output projection ----
            pending_norm = []

            def emit_norm():
                h_, qg_, ps_y_, ps_l_ = pending_norm.pop(0)
                r128 = rpool.tile([P, QG], F32, tag="r", name="r128")
                nc.vector.reciprocal_approx_fast(r128[:], ps_l_[:])
                if qg_ == 0:
                    nc.vector.tensor_mul(ybf[:, h_, :], ps_y_[:], r128[:])
                else:
                    nc.vector.tensor_mul(
                        y8t[qg_ - 1][:, h_, :], ps_y_[:], r128[:]
                    )

            def emit_outproj_tile(tg_, oc):
                ps_o = mmp.tile([P, QG], F32, tag="mm", name="ps_o")
                if tg_ == 0:
                    for hc in range(4):
                        nc.tensor.matmul(
                            ps_o[:],
                            wpbf[:, hc, oc * P:(oc + 1) * P],
                            ybf[:, hc, :],
                            start=(hc == 0),
                            stop=(hc == 3),
                        )
                else:
                    for i in range(2):
                        nc.tensor.matmul(
                            ps_o[:],
                            wp8sb[:, 2 * i:2 * i + 2, oc * P:(oc + 1) * P],
                            y8t[tg_ - 1][:, 2 * i:2 * i + 2, :],
                            start=(i == 0),
                            stop=(i == 1),
                            perf_mode=DR,
                        )
                stage = stp.tile([P, QG], BF16, tag="s", name="stage")
                if tg_ == 0:
                    nc.vector.tensor_copy(stage[:], ps_o[:])
                else:
                    nc.vector.tensor_scalar_mul(stage[:], ps_o[:], 1.0 / (SY * SW))
                nc.sync.dma_start(
                    outT[oc * P:(oc + 1) * P, tg_ * QG:(tg_ + 1) * QG], stage[:]
                )

            op_queue = []  # deferred outproj tiles, emitted during next qg

            def attn(h, qg):
                n_kt = 4 * (qg + 1)
                qsl = slice(qg * QG, (qg + 1) * QG)
                ps_y = ytp.tile([P, QG], F32, tag="yt", name="ps_y")
                ps_l = lpp.tile([P, QG], F32, tag="l", name="ps_l")
                LA = 3
                p_cur = {}   # fp8: pair index -> p2 tile; bf16: kt -> tile

                def emit_s(kt):
                    ps_s = mmp.tile([P, QG], F32, tag="mm", name="ps_s")
                    if qg == 0:
                        nc.tensor.matmul(
                            ps_s[:],
                            qkbf[:, 4 + h, kt * P:(kt + 1) * P],
                            qkbf[:, h, :],
                            start=True,
                            stop=True,
                        )
                        dst = ppb.tile([P, QG], BF16, tag="pb", name="pb")
                        p_cur[kt] = dst
                        dsl = dst[:]
                    else:
                        if kt % 2 == 0:
                            p_cur[kt // 2] = pp8.tile(
                                [P, 2, QG], FP8, tag="p2", name="p2"
                            )
                        nc.tensor.matmul(
                            ps_s[:],
                            k8[h][:, kt * P:(kt + 1) * P],
                            q8[h][:, qsl],
                            start=True,
                            stop=True,
                        )
                        dsl = p_cur[kt // 2][:, kt % 2, :]
                    nc.scalar.activation(dsl, ps_s[:], AF.Exp, bias=ebias[:], scale=SC8)
                    if kt >= 4 * qg:
                        s = kt - 4 * qg
                        nc.vector.tensor_mul(dsl, dsl, mk8[:, s, :])

                def emit_av(t):
                    if qg == 0:
                        p = p_cur.pop(t)
                        nc.tensor.matmul(
                            ps_y[:],
                            vbf[:, t, h * P:(h + 1) * P],
                            p[:],
                            start=(t == 0),
                            stop=(t == n_kt - 1),
                        )
                        nc.tensor.matmul(
                            ps_l[:],
                            onesbf[:],
                            p[:],
                            start=(t == 0),
                            stop=(t == n_kt - 1),
                        )
                    else:
                        if t % 2 == 0:
                            return
                        pr = t // 2
                        p2 = p_cur.pop(pr)
                        nc.tensor.matmul(
                            ps_y[:],
                            v8[:, 2 * pr:2 * pr + 2, h * P:(h + 1) * P],
                            p2[:],
                            start=(pr == 0),
                            stop=(t == n_kt - 1),
                            perf_mode=DR,
                        )
                        nc.tensor.matmul(
                            ps_l[:],
                            onesdr[:],
                            p2[:],
                            start=(pr == 0),
                            stop=(t == n_kt - 1),
                            perf_mode=DR,
                        )

                for step in range(n_kt + LA):
                    if step < n_kt:
                        emit_s(step)
                    if step == 2 and pending_norm:
                        emit_norm()
                    if step >= LA:
                        emit_av(step - LA)
                pending_norm.append((h, qg, ps_y, ps_l))

            for qg in range(TG):
                for h in range(4):
                    attn(h, qg)
                    for _ in range(4):
                        if op_queue:
                            op_queue.pop(0)()
                while pending_norm:
                    emit_norm()
                g = qg
                op_queue.extend(
                    (lambda tg_=g, oc_=oc: emit_outproj_tile(tg_, oc_))
                    for oc in range(16)
                )
            while op_queue:
                op_queue.pop(0)()

    nc.finalize()
    return nc


def _host_inputs(x, freqs_cis, w_attn, w_proj):
    """Build the 8 per-core input maps."""
    x = np.asarray(x, dtype=np.float32)
    freqs_cis = np.asarray(freqs_cis, dtype=np.float32)
    w_attn = np.asarray(w_attn, dtype=np.float32)
    w_proj = np.asarray(w_proj, dtype=np.float32)

    B = x.shape[0]
    perm = np.concatenate([np.arange(0, HSIZE, 2), np.arange(1, HSIZE, 2)])

    cos = np.ascontiguousarray(freqs_cis[:, :, 0].T)  # [64, T]
    sin = np.ascontiguousarray(freqs_cis[:, :, 1].T)
    c1 = np.concatenate([cos, cos], axis=0).astype(BF)   # [128, T]
    c2 = np.concatenate([-sin, sin], axis=0).astype(BF)

    kk = np.arange(P)[:, None]
    ccol = np.arange(QG)[None, :]
    mk = np.stack(
        [(ccol >= s * 128 + kk).astype(np.float32) for s in range(4)], axis=0
    )  # [4,128,512]
    mk8 = mk.astype(E4)

    onesdr = (np.full((P, 2, P), 0.5, np.float32)).astype(E4)
    onesbf = np.ones((P, P), np.float32).astype(BF)
    swp = np.zeros((P, P), np.float32)
    for m in range(P):
        swp[(m + 64) % P, m] = 1.0
    swp8 = swp.astype(E4)
    swpbf_h = swp.astype(BF)

    xT = [np.ascontiguousarray(x[b].T) for b in range(B)]
    x8 = [t.astype(E4) for t in xT]
    xbf = [np.ascontiguousarray(t[:, :QG]).astype(BF) for t in xT]

    in_maps = []
    for core in range(N_CORES):
        b, g = core // 4, core % 4
        qk_blocks = []
        for off in (0, C):  # q then k
            for hh in range(4):
                hglob = 4 * g + hh
                cols = w_attn[:, off + hglob * HSIZE: off + (hglob + 1) * HSIZE]
                qk_blocks.append(cols[:, perm])
        wqk = np.concatenate(qk_blocks, axis=1)          # [C, 1024]
        w8 = (SW * wqk).astype(E4)
        wqkbf_h = (SW * wqk).astype(BF)
        wv = w_attn[:, 2 * C + 4 * g * HSIZE: 2 * C + 4 * (g + 1) * HSIZE]
        wv8 = (SW * wv).astype(E4)
        wvbf = wv.astype(BF)
        wp = w_proj[g * 512:(g + 1) * 512, :]
        wp8 = (SW * wp).astype(E4)
        wpbf = wp.astype(BF)
        in_maps.append(
            {
                "x8": x8[b],
                "xbf": xbf[b],
                "w8": np.ascontiguousarray(w8),
                "wqkbf": np.ascontiguousarray(wqkbf_h),
                "wv8": np.ascontiguousarray(wv8),
                "wvbf": np.ascontiguousarray(wvbf),
                "wp8": np.ascontiguousarray(wp8),
                "wpbf": np.ascontiguousarray(wpbf),
                "c1": c1,
                "c2": c2,
                "mk8": mk8,
                "onesdr": onesdr,
                "onesbf": onesbf,
                "swp8": swp8,
                "swpbf": swpbf_h,
            }
        )
    return in_maps


_LAST_RESULT = {}


def kernel(x, freqs_cis, w_attn, w_proj):
    if _TRACE:
        _install_ntff_hook()
    in_maps = _host_inputs(x, freqs_cis, w_attn, w_proj)
    nc = build_nc()
    res = run_bass_kernel_spmd(
        nc, in_maps, core_ids=list(range(N_CORES)), trace=_TRACE
    )
    _LAST_RESULT["res"] = res

    B = x.shape[0]
    out = np.zeros((B, T, C), dtype=np.float32)
    for core in range(N_CORES):
        b = core // 4
        out[b] += np.asarray(res.results[core]["outT"]).astype(np.float32).T
    return out
